# revision 26
# baseline (speedup 1.0000x reference)
"""Causal multi-head self-attention with interleaved RoPE on 8 Trainium2 cores.

Problem: B=4, S=2048, D=2048, H=16 (head dim 128), fp32 in/out.

Sharding: core c handles batch b=c//2 and head-half hh=c%2 (8 heads).
Wq/Wk/Wv are column-parallel (head dim), Wo row-parallel; the host sums the
two partial outputs per batch.

Per-core dataflow (all matmuls bf16 operands, fp32 PSUM accumulation):
  A) q,k projection: out[s,o] tiles via lhsT=x^T chunks, rhs=W^T; RoPE applied
     in [s,d] layout using an evens-first permutation of the head dim (host
     permutes Wq/Wk rows so even/odd rotation pairs are contiguous); PE
     transposes q,k to [d,s] layout.
  B) v projection (natural [s,d] layout), with a ones column appended on
     device (col 128 of each [skv,129] chunk) for softmax denominators.
  C) attention per (head, 512-wide sq tile): scores^T = k^T-chunk.T @ q^T in
     PSUM, exp on ScalarE (no max subtraction: |scores|<=~12 so fp32 exp is
     safe), causal handled by trimming tile widths + one static 128x128
     triangular mask; PV with P^T chunks stationary against v_aug gives
     out[sq,d] plus the denominator column; normalize via per-partition
     reciprocal scale on ScalarE; PE-transpose to out^T[d,sq].
  D) y = out^T.T @ Wo^T rows, accumulated over 8 heads, fp32 out.
"""

import numpy as np
import ml_dtypes

B, S, D, H = 4, 2048, 2048, 16
DK = 128
HL = 8          # heads per core
NCORES = 8
P = 128
NIC = D // P    # 16 contraction chunks
NSC = S // P    # 16 s-chunks of 128
SCALE = 1.0 / float(np.sqrt(DK))
BF16 = ml_dtypes.bfloat16

_RUNNER_CACHE = {}


def _build_nc(repeat=1):
    from contextlib import ExitStack

    from concourse import bacc
    import concourse.tile as tile
    import concourse.mybir as mybir

    dt = mybir.dt
    AF = mybir.ActivationFunctionType

    nc = bacc.Bacc("TRN2", target_bir_lowering=False, debug=False)

    xt = nc.dram_tensor("xt", [P, S // 256, NIC, 256], dt.bfloat16,
                    kind="ExternalInput")
    wqkvt = nc.dram_tensor("wqkvt", [D, 3 * HL * DK], dt.bfloat16,
                           kind="ExternalInput")
    wot = nc.dram_tensor("wot", [HL * DK, D], dt.bfloat16, kind="ExternalInput")
    cosb = nc.dram_tensor("cosb", [S, 64], dt.bfloat16, kind="ExternalInput")
    sinb = nc.dram_tensor("sinb", [S, 64], dt.bfloat16, kind="ExternalInput")
    tri = nc.dram_tensor("tri", [P, P], dt.bfloat16, kind="ExternalInput")
    iden = nc.dram_tensor("iden", [P, P], dt.bfloat16, kind="ExternalInput")
    y = nc.dram_tensor("y", [S, D], dt.float32, kind="ExternalOutput")


    with tile.TileContext(nc) as tc, ExitStack() as top:
        const_pool = top.enter_context(tc.tile_pool(name="const", bufs=1))
        tri_t = const_pool.tile([P, P], dt.bfloat16)
        nc.sync.dma_start(tri_t[:], tri[:])
        iden_t = const_pool.tile([P, P], dt.bfloat16)
        nc.sync.dma_start(iden_t[:], iden[:])

        persist = top.enter_context(tc.tile_pool(name="persist", bufs=1))
        qT = persist.tile([P, HL, S], dt.bfloat16)          # [d, h, s]
        kT = persist.tile([P, HL, S], dt.bfloat16)          # [d, h, s]
        vS = persist.tile([P, NSC, HL, DK + 2], dt.bfloat16)  # [s_in, sc, h, d+ones]


        # ones column for the denominator (cols 128.. of each v chunk)
        nc.vector.memset(vS[:, :, :, DK:], 1.0)

        from contextlib import nullcontext
        loop_cm = tc.For_i(0, repeat, 1) if repeat > 1 else nullcontext()
        with loop_cm:
            # -------- Phase A+B: q,k,v projection + RoPE + transposes ----
            # Two passes of 4 heads each; one x^T LDW feeds 3 matmuls
            # (q, k, v blocks of 512 = 4 heads).
            with ExitStack() as ctxA:
                cpool = ctxA.enter_context(tc.tile_pool(name="cossin", bufs=1))
                cos_t = cpool.tile([P, NSC, 64], dt.bfloat16)
                nc.gpsimd.dma_start(
                    cos_t[:], cosb[:].rearrange("(sc p) j -> p sc j", p=P))
                sin_t = cpool.tile([P, NSC, 64], dt.bfloat16)
                nc.gpsimd.dma_start(
                    sin_t[:], sinb[:].rearrange("(sc p) j -> p sc j", p=P))

                for g in range(2):                # head groups 0-3, 4-7
                    with ExitStack() as ctx2:
                        wpool = ctx2.enter_context(tc.tile_pool(name="wg", bufs=1))
                        xpool = ctx2.enter_context(tc.tile_pool(name="xa", bufs=2))
                        psA = ctx2.enter_context(
                            tc.tile_pool(name="psA", bufs=2, space="PSUM"))
                        psT = ctx2.enter_context(
                            tc.tile_pool(name="psT", bufs=2, space="PSUM"))
                        rpool = ctx2.enter_context(tc.tile_pool(name="rope", bufs=2))

                        w_c = [wpool.tile([P, 1536], dt.bfloat16,
                                          tag=f"w{ic}", name=f"w{ic}")
                               for ic in range(NIC)]
                        xt_first = xpool.tile([P, NIC, 256], dt.bfloat16, tag="xt")
                        nc.sync.dma_start(xt_first[:], xt[:, 0, :, :])
                        for ic in range(NIC):
                            nc.gpsimd.dma_start(
                                w_c[ic][:],
                                wqkvt[ic * P:(ic + 1) * P,
                                      g * 1536:(g + 1) * 1536])

                        for so in range(S // 256):    # x chunks of 256
                            if so == 0:
                                xt_t = xt_first
                            else:
                                xt_t = xpool.tile([P, NIC, 256], dt.bfloat16,
                                                  tag="xt")
                                nc.sync.dma_start(xt_t[:], xt[:, so, :, :])
                            for ss in range(2):       # s-subchunk of 128
                                sc = so * 2 + ss
                                pq = [psA.tile([P, 512], dt.float32, tag=f"pa{j}",
                                               name=f"pa{j}")
                                      for j in range(3)]
                                for ic in range(NIC):
                                    lw = xt_t[:, ic, ss * 128:(ss + 1) * 128]
                                    for j in range(3):
                                        nc.tensor.matmul(
                                            pq[j][:], lw,
                                            w_c[ic][:, j * 512:(j + 1) * 512],
                                            start=(ic == 0), stop=(ic == NIC - 1))
                                # v block straight to vS (natural [s, d] layout)
                                nc.any.tensor_copy(
                                    out=vS[:, sc, g * 4:(g + 1) * 4, 0:DK],
                                    in_=pq[2][:].rearrange("p (t d) -> p t d", d=P))
                                # q,k blocks: copy, RoPE, transpose
                                qk_s = rpool.tile([P, 8, P], dt.bfloat16, tag="qks")
                                for j in range(2):
                                    nc.any.tensor_copy(
                                        out=qk_s[:, j * 4:(j + 1) * 4, :],
                                        in_=pq[j][:].rearrange(
                                            "p (t d) -> p t d", d=P))
                                # RoPE (evens-first: E=cols 0:64, O=cols 64:128)
                                E = qk_s[:, :, 0:64]
                                O = qk_s[:, :, 64:128]
                                cB = cos_t[:, sc, None, :].to_broadcast((P, 8, 64))
                                sB = sin_t[:, sc, None, :].to_broadcast((P, 8, 64))
                                t1 = rpool.tile([P, 8, 64], dt.bfloat16, tag="t1")
                                t2 = rpool.tile([P, 8, 64], dt.bfloat16, tag="t2")
                                rot = rpool.tile([P, 8, P], dt.bfloat16, tag="rot")
                                nc.vector.tensor_mul(out=t1[:], in0=E, in1=cB)
                                nc.vector.tensor_mul(out=t2[:], in0=O, in1=sB)
                                nc.vector.tensor_sub(out=rot[:, :, 0:64],
                                                     in0=t1[:], in1=t2[:])
                                nc.vector.tensor_mul(out=t1[:], in0=O, in1=cB)
                                nc.vector.tensor_mul(out=t2[:], in0=E, in1=sB)
                                nc.vector.tensor_add(out=rot[:, :, 64:128],
                                                     in0=t1[:], in1=t2[:])
                                for tt in range(8):
                                    pst = psT.tile([P, P], dt.bfloat16, tag="ptr")
                                    nc.tensor.transpose(pst[:], rot[:, tt, :],
                                                        iden_t[:])
                                    dst = qT if tt < 4 else kT
                                    nc.any.tensor_copy(
                                        out=dst[:, g * 4 + tt % 4,
                                                sc * 128:(sc + 1) * 128],
                                        in_=pst[:])

            # ---------------- Phase C: attention ---------------------------
            with ExitStack() as ctxCD:
                opool = ctxCD.enter_context(tc.tile_pool(name="oTp", bufs=1))
                oTt = [opool.tile([P, HL, 512], dt.bfloat16, tag=f"oT{t}",
                                  name=f"oT{t}")
                       for t in range(S // 512)]  # attn out^T [d, h, s] per t
                # prefetch Wo during attention
                wpool = ctxCD.enter_context(tc.tile_pool(name="wo", bufs=1))
                wo_c = [wpool.tile([P, D], dt.bfloat16, tag=f"wo{hh}",
                                   name=f"wo{hh}") for hh in range(HL)]
                dma_engs = [nc.gpsimd, nc.gpsimd]
                for hh in range(HL):
                    dma_engs[hh % 2].dma_start(wo_c[hh][:],
                                               wot[hh * P:(hh + 1) * P, :])
                ctx = ctxCD.enter_context(ExitStack())
                psS = ctx.enter_context(tc.tile_pool(name="psS", bufs=2, space="PSUM"))
                psO = ctx.enter_context(tc.tile_pool(name="psO", bufs=1, space="PSUM"))
                psT2 = ctx.enter_context(tc.tile_pool(name="psT2", bufs=1, space="PSUM"))
                apool = ctx.enter_context(tc.tile_pool(name="attn", bufs=4))
                npool = ctx.enter_context(tc.tile_pool(name="norm", bufs=2))

                SQT = 512                         # sq tile width
                NSS = SQT // 128                  # 4 sq-subchunks / tile
                psD = ctx.enter_context(
                    tc.tile_pool(name="psD", bufs=1, space="PSUM"))
                ypool = ctx.enter_context(tc.tile_pool(name="ysb", bufs=2))

                def emit_d(t, sl):
                    """Output-projection chunk for s rows [512t+128sl, +128).

                    Emitted interleaved with the next tile-row's attention so
                    its matmuls fill PE idle while ScalarE runs exp.
                    """
                    oT = oTt[t]
                    sc = t * NSS + sl
                    y_sb = ypool.tile([P, D], dt.float32, tag="ysb")
                    for j in range(4):
                        yp = psD.tile([P, 512], dt.float32, tag="pd", name="pd")
                        for h in range(HL):
                            nc.tensor.matmul(
                                yp[:], oT[:, h, sl * 128:(sl + 1) * 128],
                                wo_c[h][:, j * 512:(j + 1) * 512],
                                start=(h == 0), stop=(h == HL - 1))
                        nc.any.tensor_copy(out=y_sb[:, j * 512:(j + 1) * 512],
                                           in_=yp[:])
                    nc.sync.dma_start(y[sc * P:(sc + 1) * P, :], y_sb[:])

                for t in range(S // SQT):
                    oT = oTt[t]
                    for h in range(HL):
                        out_ps = [psO.tile([P, DK + 2], dt.float32,
                                           tag=f"outp{b}", name=f"outp{b}")
                                  for b in range(NSS)]
                        nkv = NSS * (t + 1)
                        for c in range(nkv):
                            r = c - NSS * t       # >=0: diagonal-region chunk
                            sq_off = 128 * r if r > 0 else 0
                            width = SQT - sq_off
                            q0 = t * SQT + sq_off
                            sc_ps = psS.tile([P, SQT], dt.float32, tag="sc")
                            nc.tensor.matmul(
                                sc_ps[:, :width],
                                kT[:, h, c * 128:(c + 1) * 128],
                                qT[:, h, q0:q0 + width],
                                start=True, stop=True)
                            pt = apool.tile([P, SQT], dt.bfloat16, tag="pt")
                            nc.scalar.activation(pt[:, :width], sc_ps[:, :width],
                                                 AF.Exp, scale=SCALE)
                            if r >= 0:
                                pt_m = apool.tile([P, P], dt.bfloat16, tag="ptm")
                                nc.vector.tensor_mul(out=pt_m[:],
                                                     in0=pt[:, 0:128], in1=tri_t[:])
                            for ss in range(max(r, 0), NSS):
                                lo = ss * 128 - sq_off
                                lhsT = pt_m[:] if (r >= 0 and ss == r) \
                                    else pt[:, lo:lo + 128]
                                nc.tensor.matmul(
                                    out_ps[ss], lhsT,
                                    vS[:, c, h, :],
                                    start=(c == 0), stop=(c == NSS * t + ss))
                        # evict accumulators fast (frees PSUM), then normalize
                        for ss in range(NSS):
                            raw = npool.tile([P, DK + 2], dt.float32, tag="raw")
                            nc.vector.tensor_copy(raw[:], out_ps[ss][:])
                            rec = npool.tile([P, 1], dt.float32, tag="rec")
                            nc.vector.reciprocal(rec[:], raw[:, DK:DK + 1])
                            ob = npool.tile([P, DK], dt.bfloat16, tag="ob")
                            nc.vector.tensor_scalar_mul(ob[:], raw[:, 0:DK],
                                                        rec[:])
                            pst = psT2.tile([P, P], dt.bfloat16, tag="ptr2")
                            nc.tensor.transpose(pst[:], ob[:], iden_t[:])
                            nc.vector.tensor_copy(oT[:, h, ss * 128:(ss + 1) * 128],
                                                  pst[:])
                        # interleave previous tile-row's output projection
                        if t >= 1 and h % 2 == 1:
                            emit_d(t - 1, h // 2)
                    if t == S // SQT - 1:
                        for sl in range(NSS):
                            emit_d(t, sl)

    nc.finalize()
    return nc


def _make_runner(nc, n_cores=NCORES):
    import jax
    from jax.sharding import Mesh, PartitionSpec
    from jax.experimental.shard_map import shard_map

    import concourse.mybir as mybir_mod
    import concourse.mybir as mybir
    from concourse import bass2jax
    from concourse.bass2jax import _bass_exec_p, install_neuronx_cc_hook

    install_neuronx_cc_hook()
    in_names, out_names, out_avals = [], [], []
    partition_name = nc.partition_id_tensor.name if nc.partition_id_tensor else None
    for alloc in nc.m.functions[0].allocations:
        if not isinstance(alloc, mybir_mod.MemoryLocationSet):
            continue
        name = alloc.memorylocations[0].name
        if alloc.kind == "ExternalInput":
            if name != partition_name:
                in_names.append(name)
        elif alloc.kind == "ExternalOutput":
            out_names.append(name)
            out_avals.append(jax.core.ShapedArray(
                tuple(alloc.tensor_shape), mybir.dt.np(alloc.dtype)))
    n_params = len(in_names)
    all_in_names = list(in_names) + list(out_names)
    if partition_name is not None:
        all_in_names.append(partition_name)

    def _body(*args):
        operands = list(args)
        if partition_name is not None:
            operands.append(bass2jax.partition_id_tensor())
        outs = _bass_exec_p.bind(
            *operands,
            out_avals=tuple(out_avals),
            in_names=tuple(all_in_names),
            out_names=tuple(out_names),
            lowering_input_output_aliases=(),
            sim_require_finite=True,
            sim_require_nnan=True,
            nc=nc,
        )
        return tuple(outs)

    devices = jax.devices()[:n_cores]
    mesh = Mesh(np.asarray(devices), ("core",))
    in_specs = (PartitionSpec("core"),) * (n_params + len(out_names))
    out_specs = (PartitionSpec("core"),) * len(out_names)
    fn = jax.jit(shard_map(_body, mesh=mesh, in_specs=in_specs,
                           out_specs=out_specs, check_rep=False))
    zero_outs = [np.zeros((n_cores * a.shape[0],) + tuple(a.shape[1:]), a.dtype)
                 for a in out_avals]
    return fn, in_names, out_names, zero_outs


def _get_runner(repeat=1):
    if repeat not in _RUNNER_CACHE:
        nc = _build_nc(repeat)
        _RUNNER_CACHE[repeat] = _make_runner(nc)
    return _RUNNER_CACHE[repeat]


def _prep_in_maps(x, Wq, Wk, Wv, Wo, cos, sin):
    x = np.asarray(x, dtype=np.float32)
    Wq = np.asarray(Wq, dtype=np.float32)
    Wk = np.asarray(Wk, dtype=np.float32)
    Wv = np.asarray(Wv, dtype=np.float32)
    Wo = np.asarray(Wo, dtype=np.float32)
    cos = np.asarray(cos, dtype=np.float32)
    sin = np.asarray(sin, dtype=np.float32)

    # evens-first permutation of the head dim for q/k (RoPE pair layout)
    perm = np.concatenate([np.arange(0, DK, 2), np.arange(1, DK, 2)])
    rows = (np.arange(H)[:, None] * DK + perm[None, :]).reshape(-1)
    Wq_p = Wq[rows]
    Wk_p = Wk[rows]

    cosb = cos.astype(BF16)
    sinb = sin.astype(BF16)
    tri = (np.arange(P)[:, None] <= np.arange(P)[None, :]).astype(BF16)
    iden = np.eye(P, dtype=np.float32).astype(BF16)

    in_maps = []
    for c in range(NCORES):
        b, hh = divmod(c, 2)
        osl = slice(hh * HL * DK, (hh + 1) * HL * DK)
        xt = x[b].T.reshape(NIC, P, S // 256, 256).transpose(1, 2, 0, 3)
        g0 = slice(hh * HL * DK, hh * HL * DK + 4 * DK)
        g1 = slice(hh * HL * DK + 4 * DK, (hh + 1) * HL * DK)
        wqkvt = np.concatenate(
            [Wq_p[g0].T, Wk_p[g0].T, Wv[g0].T,
             Wq_p[g1].T, Wk_p[g1].T, Wv[g1].T], axis=1)
        in_maps.append({
            "xt": np.ascontiguousarray(xt).astype(BF16),
            "wqkvt": wqkvt.astype(BF16),
            "wot": np.ascontiguousarray(Wo[:, osl].T).astype(BF16),
            "cosb": cosb,
            "sinb": sinb,
            "tri": tri,
            "iden": iden,
        })
    return in_maps


def _run(in_maps, repeat=1):
    import jax

    fn, in_names, out_names, zero_outs = _get_runner(repeat)
    concat_in = [np.concatenate([m[name] for m in in_maps], axis=0)
                 for name in in_names]
    out_arrs = fn(*concat_in, *zero_outs)
    yname = out_names.index("y")
    yall = np.asarray(out_arrs[yname]).reshape(NCORES, S, D)
    return yall


def kernel(x, Wq, Wk, Wv, Wo, cos, sin):
    in_maps = _prep_in_maps(x, Wq, Wk, Wv, Wo, cos, sin)
    yall = _run(in_maps)
    out = np.empty((B, S, D), dtype=np.float32)
    for b in range(B):
        out[b] = yall[2 * b] + yall[2 * b + 1]
    return out


# revision 29
# speedup vs baseline: 1.0250x; 1.0250x over previous
"""Causal multi-head self-attention with interleaved RoPE on 8 Trainium2 cores.

Problem: B=4, S=2048, D=2048, H=16 (head dim 128), fp32 in/out.

Sharding: core c handles batch b=c//2 and head-half hh=c%2 (8 heads).
Wq/Wk/Wv are column-parallel (head dim), Wo row-parallel; the host sums the
two partial outputs per batch.

Per-core dataflow (all matmuls bf16 operands, fp32 PSUM accumulation):
  A) q,k projection: out[s,o] tiles via lhsT=x^T chunks, rhs=W^T; RoPE applied
     in [s,d] layout using an evens-first permutation of the head dim (host
     permutes Wq/Wk rows so even/odd rotation pairs are contiguous); PE
     transposes q,k to [d,s] layout.
  B) v projection (natural [s,d] layout), with a ones column appended on
     device (col 128 of each [skv,129] chunk) for softmax denominators.
  C) attention per (head, 512-wide sq tile): scores^T = k^T-chunk.T @ q^T in
     PSUM, exp on ScalarE (no max subtraction: |scores|<=~12 so fp32 exp is
     safe), causal handled by trimming tile widths + one static 128x128
     triangular mask; PV with P^T chunks stationary against v_aug gives
     out[sq,d] plus the denominator column; normalize via per-partition
     reciprocal scale on ScalarE; PE-transpose to out^T[d,sq].
  D) y = out^T.T @ Wo^T rows, accumulated over 8 heads, fp32 out.
"""

import numpy as np
import ml_dtypes

B, S, D, H = 4, 2048, 2048, 16
DK = 128
HL = 8          # heads per core
NCORES = 8
P = 128
NIC = D // P    # 16 contraction chunks
NSC = S // P    # 16 s-chunks of 128
SCALE = 1.0 / float(np.sqrt(DK))
BF16 = ml_dtypes.bfloat16

_RUNNER_CACHE = {}
_PHASES = "abcd"   # debug: truncate kernel for phase timing


def _build_nc(repeat=1, phases=None):
    if phases is None:
        phases = _PHASES
    from contextlib import ExitStack

    from concourse import bacc
    import concourse.tile as tile
    import concourse.mybir as mybir

    dt = mybir.dt
    AF = mybir.ActivationFunctionType

    nc = bacc.Bacc("TRN2", target_bir_lowering=False, debug=False)

    xt = nc.dram_tensor("xt", [P, S // 256, NIC, 256], dt.bfloat16,
                    kind="ExternalInput")
    wqkvt = nc.dram_tensor("wqkvt", [D, 3 * HL * DK], dt.bfloat16,
                           kind="ExternalInput")
    wot = nc.dram_tensor("wot", [HL * DK, D], dt.bfloat16, kind="ExternalInput")
    cosb = nc.dram_tensor("cosb", [S, 64], dt.bfloat16, kind="ExternalInput")
    sinb = nc.dram_tensor("sinb", [S, 64], dt.bfloat16, kind="ExternalInput")
    tri = nc.dram_tensor("tri", [P, P], dt.bfloat16, kind="ExternalInput")
    iden = nc.dram_tensor("iden", [P, P], dt.bfloat16, kind="ExternalInput")
    y = nc.dram_tensor("y", [S, D], dt.float32, kind="ExternalOutput")


    with tile.TileContext(nc) as tc, ExitStack() as top:
        const_pool = top.enter_context(tc.tile_pool(name="const", bufs=1))
        tri_t = const_pool.tile([P, P], dt.bfloat16)
        nc.sync.dma_start(tri_t[:], tri[:])
        iden_t = const_pool.tile([P, P], dt.bfloat16)
        nc.sync.dma_start(iden_t[:], iden[:])

        persist = top.enter_context(tc.tile_pool(name="persist", bufs=1))
        qT = persist.tile([P, HL, S], dt.bfloat16)          # [d, h, s]
        kT = persist.tile([P, HL, S], dt.bfloat16)          # [d, h, s]
        vS = persist.tile([P, NSC, HL, DK + 2], dt.bfloat16)  # [s_in, sc, h, d+ones]


        # ones column for the denominator (cols 128.. of each v chunk)
        nc.vector.memset(vS[:, :, :, DK:], 1.0)

        from contextlib import nullcontext
        loop_cm = tc.For_i(0, repeat, 1) if repeat > 1 else nullcontext()
        with loop_cm:
            # -------- Phase A+B: q,k,v projection + RoPE + transposes ----
            # Two passes of 4 heads each; one x^T LDW feeds 3 matmuls
            # (q, k, v blocks of 512 = 4 heads).
            with ExitStack() as ctxA:
                cpool = ctxA.enter_context(tc.tile_pool(name="cossin", bufs=1))
                cos_t = cpool.tile([P, NSC, 64], dt.bfloat16)
                nc.gpsimd.dma_start(
                    cos_t[:], cosb[:].rearrange("(sc p) j -> p sc j", p=P))
                sin_t = cpool.tile([P, NSC, 64], dt.bfloat16)
                nc.gpsimd.dma_start(
                    sin_t[:], sinb[:].rearrange("(sc p) j -> p sc j", p=P))

                for g in range(2):                # head groups 0-3, 4-7
                    with ExitStack() as ctx2:
                        wpool = ctx2.enter_context(tc.tile_pool(name="wg", bufs=1))
                        xpool = ctx2.enter_context(tc.tile_pool(name="xa", bufs=2))
                        psA = ctx2.enter_context(
                            tc.tile_pool(name="psA", bufs=2, space="PSUM"))
                        psT = ctx2.enter_context(
                            tc.tile_pool(name="psT", bufs=2, space="PSUM"))
                        rpool = ctx2.enter_context(tc.tile_pool(name="rope", bufs=2))

                        w_c = [wpool.tile([P, 1536], dt.bfloat16,
                                          tag=f"w{ic}", name=f"w{ic}")
                               for ic in range(NIC)]
                        xt_first = xpool.tile([P, NIC, 256], dt.bfloat16, tag="xt")
                        nc.sync.dma_start(xt_first[:], xt[:, 0, :, :])
                        for ic in range(NIC):
                            nc.gpsimd.dma_start(
                                w_c[ic][:],
                                wqkvt[ic * P:(ic + 1) * P,
                                      g * 1536:(g + 1) * 1536])

                        for so in range(S // 256):    # x chunks of 256
                            if so == 0:
                                xt_t = xt_first
                            else:
                                xt_t = xpool.tile([P, NIC, 256], dt.bfloat16,
                                                  tag="xt")
                                nc.sync.dma_start(xt_t[:], xt[:, so, :, :])
                            for ss in range(2):       # s-subchunk of 128
                                sc = so * 2 + ss
                                pq = [psA.tile([P, 512], dt.float32, tag=f"pa{j}",
                                               name=f"pa{j}")
                                      for j in range(3)]
                                for ic in range(NIC):
                                    lw = xt_t[:, ic, ss * 128:(ss + 1) * 128]
                                    for j in range(3):
                                        nc.tensor.matmul(
                                            pq[j][:], lw,
                                            w_c[ic][:, j * 512:(j + 1) * 512],
                                            start=(ic == 0), stop=(ic == NIC - 1))
                                # v block straight to vS (natural [s, d] layout)
                                nc.any.tensor_copy(
                                    out=vS[:, sc, g * 4:(g + 1) * 4, 0:DK],
                                    in_=pq[2][:].rearrange("p (t d) -> p t d", d=P))
                                # q,k blocks: copy, RoPE, transpose
                                qk_s = rpool.tile([P, 8, P], dt.bfloat16, tag="qks")
                                for j in range(2):
                                    nc.any.tensor_copy(
                                        out=qk_s[:, j * 4:(j + 1) * 4, :],
                                        in_=pq[j][:].rearrange(
                                            "p (t d) -> p t d", d=P))
                                # RoPE (evens-first: E=cols 0:64, O=cols 64:128)
                                E = qk_s[:, :, 0:64]
                                O = qk_s[:, :, 64:128]
                                cB = cos_t[:, sc, None, :].to_broadcast((P, 8, 64))
                                sB = sin_t[:, sc, None, :].to_broadcast((P, 8, 64))
                                t1 = rpool.tile([P, 8, 64], dt.bfloat16, tag="t1")
                                t2 = rpool.tile([P, 8, 64], dt.bfloat16, tag="t2")
                                rot = rpool.tile([P, 8, P], dt.bfloat16, tag="rot")
                                nc.vector.tensor_mul(out=t1[:], in0=E, in1=cB)
                                nc.vector.tensor_mul(out=t2[:], in0=O, in1=sB)
                                nc.vector.tensor_sub(out=rot[:, :, 0:64],
                                                     in0=t1[:], in1=t2[:])
                                nc.vector.tensor_mul(out=t1[:], in0=O, in1=cB)
                                nc.vector.tensor_mul(out=t2[:], in0=E, in1=sB)
                                nc.vector.tensor_add(out=rot[:, :, 64:128],
                                                     in0=t1[:], in1=t2[:])
                                for tt in range(8):
                                    pst = psT.tile([P, P], dt.bfloat16, tag="ptr")
                                    nc.tensor.transpose(pst[:], rot[:, tt, :],
                                                        iden_t[:])
                                    dst = qT if tt < 4 else kT
                                    nc.any.tensor_copy(
                                        out=dst[:, g * 4 + tt % 4,
                                                sc * 128:(sc + 1) * 128],
                                        in_=pst[:])

            # ---------------- Phase C: attention ---------------------------
            with ExitStack() as ctxCD:
                opool = ctxCD.enter_context(tc.tile_pool(name="oTp", bufs=1))
                oTt = [opool.tile([P, HL, 512], dt.bfloat16, tag=f"oT{t}",
                                  name=f"oT{t}")
                       for t in range(S // 512)]  # attn out^T [d, h, s] per t
                # prefetch Wo during attention
                wpool = ctxCD.enter_context(tc.tile_pool(name="wo", bufs=1))
                wo_c = [wpool.tile([P, D], dt.bfloat16, tag=f"wo{hh}",
                                   name=f"wo{hh}") for hh in range(HL)]
                dma_engs = [nc.gpsimd, nc.gpsimd]
                for hh in range(HL):
                    dma_engs[hh % 2].dma_start(wo_c[hh][:],
                                               wot[hh * P:(hh + 1) * P, :])
                ctx = ctxCD.enter_context(ExitStack())
                psS = ctx.enter_context(tc.tile_pool(name="psS", bufs=2, space="PSUM"))
                psO = ctx.enter_context(tc.tile_pool(name="psO", bufs=1, space="PSUM"))
                psT2 = ctx.enter_context(tc.tile_pool(name="psT2", bufs=1, space="PSUM"))
                apool = ctx.enter_context(tc.tile_pool(name="attn", bufs=4))
                npool = ctx.enter_context(tc.tile_pool(name="norm", bufs=2))

                SQT = 512                         # sq tile width
                NSS = SQT // 128                  # 4 sq-subchunks / tile
                psD = ctx.enter_context(
                    tc.tile_pool(name="psD", bufs=1, space="PSUM"))
                ypool = ctx.enter_context(tc.tile_pool(name="ysb", bufs=2))

                def emit_d(t, sl):
                    """Output-projection chunk for s rows [512t+128sl, +128).

                    Emitted interleaved with the next tile-row's attention so
                    its matmuls fill PE idle while ScalarE runs exp.
                    """
                    if "d" not in phases:
                        return
                    oT = oTt[t]
                    sc = t * NSS + sl
                    y_sb = ypool.tile([P, D], dt.float32, tag="ysb")
                    for j in range(4):
                        yp = psD.tile([P, 512], dt.float32, tag="pd", name="pd")
                        for h in range(HL):
                            nc.tensor.matmul(
                                yp[:], oT[:, h, sl * 128:(sl + 1) * 128],
                                wo_c[h][:, j * 512:(j + 1) * 512],
                                start=(h == 0), stop=(h == HL - 1))
                        nc.any.tensor_copy(out=y_sb[:, j * 512:(j + 1) * 512],
                                           in_=yp[:])
                    nc.sync.dma_start(y[sc * P:(sc + 1) * P, :], y_sb[:])

                for t in range(S // SQT):
                    oT = oTt[t]
                    for h in range(HL if "c" in phases else 0):
                        out_ps = [psO.tile([P, DK + 2], dt.float32,
                                           tag=f"outp{b}", name=f"outp{b}")
                                  for b in range(NSS)]
                        nkv = NSS * (t + 1)
                        for c in range(nkv):
                            r = c - NSS * t       # >=0: diagonal-region chunk
                            sq_off = 128 * r if r > 0 else 0
                            width = SQT - sq_off
                            q0 = t * SQT + sq_off
                            sc_ps = psS.tile([P, SQT], dt.float32, tag="sc")
                            nc.tensor.matmul(
                                sc_ps[:, :width],
                                kT[:, h, c * 128:(c + 1) * 128],
                                qT[:, h, q0:q0 + width],
                                start=True, stop=True)
                            pt = apool.tile([P, SQT], dt.bfloat16, tag="pt")
                            nc.scalar.activation(pt[:, :width], sc_ps[:, :width],
                                                 AF.Exp, scale=SCALE)
                            if r >= 0:
                                pt_m = apool.tile([P, P], dt.bfloat16, tag="ptm")
                                nc.vector.tensor_mul(out=pt_m[:],
                                                     in0=pt[:, 0:128], in1=tri_t[:])
                            for ss in range(max(r, 0), NSS):
                                lo = ss * 128 - sq_off
                                lhsT = pt_m[:] if (r >= 0 and ss == r) \
                                    else pt[:, lo:lo + 128]
                                nc.tensor.matmul(
                                    out_ps[ss], lhsT,
                                    vS[:, c, h, :],
                                    start=(c == 0), stop=(c == NSS * t + ss))
                        # evict accumulators fast (frees PSUM), then normalize
                        for ss in range(NSS):
                            raw = npool.tile([P, DK + 2], dt.float32, tag="raw")
                            nc.vector.tensor_copy(raw[:], out_ps[ss][:])
                            rec = npool.tile([P, 1], dt.float32, tag="rec")
                            nc.vector.reciprocal(rec[:], raw[:, DK:DK + 1])
                            ob = npool.tile([P, DK], dt.bfloat16, tag="ob")
                            nc.vector.tensor_scalar_mul(ob[:], raw[:, 0:DK],
                                                        rec[:])
                            pst = psT2.tile([P, P], dt.bfloat16, tag="ptr2")
                            nc.tensor.transpose(pst[:], ob[:], iden_t[:])
                            nc.vector.tensor_copy(oT[:, h, ss * 128:(ss + 1) * 128],
                                                  pst[:])
                        # interleave previous tile-row's output projection
                        if t >= 1 and h % 2 == 1:
                            emit_d(t - 1, h // 2)
                    if t == S // SQT - 1:
                        for sl in range(NSS):
                            emit_d(t, sl)

    nc.finalize()
    return nc


def _make_runner(nc, n_cores=NCORES):
    import jax
    from jax.sharding import Mesh, PartitionSpec
    from jax.experimental.shard_map import shard_map

    import concourse.mybir as mybir_mod
    import concourse.mybir as mybir
    from concourse import bass2jax
    from concourse.bass2jax import _bass_exec_p, install_neuronx_cc_hook

    install_neuronx_cc_hook()
    in_names, out_names, out_avals = [], [], []
    partition_name = nc.partition_id_tensor.name if nc.partition_id_tensor else None
    for alloc in nc.m.functions[0].allocations:
        if not isinstance(alloc, mybir_mod.MemoryLocationSet):
            continue
        name = alloc.memorylocations[0].name
        if alloc.kind == "ExternalInput":
            if name != partition_name:
                in_names.append(name)
        elif alloc.kind == "ExternalOutput":
            out_names.append(name)
            out_avals.append(jax.core.ShapedArray(
                tuple(alloc.tensor_shape), mybir.dt.np(alloc.dtype)))
    n_params = len(in_names)
    all_in_names = list(in_names) + list(out_names)
    if partition_name is not None:
        all_in_names.append(partition_name)

    def _body(*args):
        operands = list(args)
        if partition_name is not None:
            operands.append(bass2jax.partition_id_tensor())
        outs = _bass_exec_p.bind(
            *operands,
            out_avals=tuple(out_avals),
            in_names=tuple(all_in_names),
            out_names=tuple(out_names),
            lowering_input_output_aliases=(),
            sim_require_finite=True,
            sim_require_nnan=True,
            nc=nc,
        )
        return tuple(outs)

    devices = jax.devices()[:n_cores]
    mesh = Mesh(np.asarray(devices), ("core",))
    in_specs = (PartitionSpec("core"),) * (n_params + len(out_names))
    out_specs = (PartitionSpec("core"),) * len(out_names)
    fn = jax.jit(shard_map(_body, mesh=mesh, in_specs=in_specs,
                           out_specs=out_specs, check_rep=False))
    zero_outs = [np.zeros((n_cores * a.shape[0],) + tuple(a.shape[1:]), a.dtype)
                 for a in out_avals]
    return fn, in_names, out_names, zero_outs


def _get_runner(repeat=1):
    key = (repeat, _PHASES)
    if key not in _RUNNER_CACHE:
        nc = _build_nc(repeat)
        _RUNNER_CACHE[key] = _make_runner(nc)
    return _RUNNER_CACHE[key]


def _prep_in_maps(x, Wq, Wk, Wv, Wo, cos, sin):
    x = np.asarray(x, dtype=np.float32)
    Wq = np.asarray(Wq, dtype=np.float32)
    Wk = np.asarray(Wk, dtype=np.float32)
    Wv = np.asarray(Wv, dtype=np.float32)
    Wo = np.asarray(Wo, dtype=np.float32)
    cos = np.asarray(cos, dtype=np.float32)
    sin = np.asarray(sin, dtype=np.float32)

    # evens-first permutation of the head dim for q/k (RoPE pair layout)
    perm = np.concatenate([np.arange(0, DK, 2), np.arange(1, DK, 2)])
    rows = (np.arange(H)[:, None] * DK + perm[None, :]).reshape(-1)
    Wq_p = Wq[rows]
    Wk_p = Wk[rows]

    cosb = cos.astype(BF16)
    sinb = sin.astype(BF16)
    tri = (np.arange(P)[:, None] <= np.arange(P)[None, :]).astype(BF16)
    iden = np.eye(P, dtype=np.float32).astype(BF16)

    in_maps = []
    for c in range(NCORES):
        b, hh = divmod(c, 2)
        osl = slice(hh * HL * DK, (hh + 1) * HL * DK)
        xt = x[b].T.reshape(NIC, P, S // 256, 256).transpose(1, 2, 0, 3)
        g0 = slice(hh * HL * DK, hh * HL * DK + 4 * DK)
        g1 = slice(hh * HL * DK + 4 * DK, (hh + 1) * HL * DK)
        wqkvt = np.concatenate(
            [Wq_p[g0].T, Wk_p[g0].T, Wv[g0].T,
             Wq_p[g1].T, Wk_p[g1].T, Wv[g1].T], axis=1)
        in_maps.append({
            "xt": np.ascontiguousarray(xt).astype(BF16),
            "wqkvt": wqkvt.astype(BF16),
            "wot": np.ascontiguousarray(Wo[:, osl].T).astype(BF16),
            "cosb": cosb,
            "sinb": sinb,
            "tri": tri,
            "iden": iden,
        })
    return in_maps


def _run(in_maps, repeat=1):
    import jax

    fn, in_names, out_names, zero_outs = _get_runner(repeat)
    concat_in = [np.concatenate([m[name] for m in in_maps], axis=0)
                 for name in in_names]
    out_arrs = fn(*concat_in, *zero_outs)
    yname = out_names.index("y")
    yall = np.asarray(out_arrs[yname]).reshape(NCORES, S, D)
    return yall


def kernel(x, Wq, Wk, Wv, Wo, cos, sin):
    in_maps = _prep_in_maps(x, Wq, Wk, Wv, Wo, cos, sin)
    yall = _run(in_maps)
    out = np.empty((B, S, D), dtype=np.float32)
    for b in range(B):
        out[b] = yall[2 * b] + yall[2 * b + 1]
    return out


# revision 30
# speedup vs baseline: 1.5099x; 1.4731x over previous
"""Causal multi-head self-attention with interleaved RoPE on 8 Trainium2 cores.

Problem: B=4, S=2048, D=2048, H=16 (head dim 128), fp32 in/out.

Sharding: core c handles batch b=c//2 and head-half hh=c%2 (8 heads).
Wq/Wk/Wv are column-parallel (head dim), Wo row-parallel; the host sums the
two partial outputs per batch.

Per-core dataflow (all matmuls bf16 operands, fp32 PSUM accumulation):
  A) q,k projection: out[s,o] tiles via lhsT=x^T chunks, rhs=W^T; RoPE applied
     in [s,d] layout using an evens-first permutation of the head dim (host
     permutes Wq/Wk rows so even/odd rotation pairs are contiguous); PE
     transposes q,k to [d,s] layout.
  B) v projection (natural [s,d] layout), with a ones column appended on
     device (col 128 of each [skv,129] chunk) for softmax denominators.
  C) attention per (head, 512-wide sq tile): scores^T = k^T-chunk.T @ q^T in
     PSUM, exp on ScalarE (no max subtraction: |scores|<=~12 so fp32 exp is
     safe), causal handled by trimming tile widths + one static 128x128
     triangular mask; PV with P^T chunks stationary against v_aug gives
     out[sq,d] plus the denominator column; normalize via per-partition
     reciprocal scale on ScalarE; PE-transpose to out^T[d,sq].
  D) y = out^T.T @ Wo^T rows, accumulated over 8 heads, fp32 out.
"""

import numpy as np
import ml_dtypes

B, S, D, H = 4, 2048, 2048, 16
DK = 128
HL = 8          # heads per core
NCORES = 8
P = 128
NIC = D // P    # 16 contraction chunks
NSC = S // P    # 16 s-chunks of 128
SCALE = 1.0 / float(np.sqrt(DK))
BF16 = ml_dtypes.bfloat16

_RUNNER_CACHE = {}
_PHASES = "abcd"   # debug: truncate kernel for phase timing
_ABLATE = set()    # debug: {"rope","trans","copies"} to skip work for timing


def _build_nc(repeat=1, phases=None):
    if phases is None:
        phases = _PHASES
    ablate = _ABLATE
    from contextlib import ExitStack

    from concourse import bacc
    import concourse.tile as tile
    import concourse.mybir as mybir

    dt = mybir.dt
    AF = mybir.ActivationFunctionType

    nc = bacc.Bacc("TRN2", target_bir_lowering=False, debug=False)

    xt = nc.dram_tensor("xt", [P, S // 256, NIC, 256], dt.bfloat16,
                    kind="ExternalInput")
    wqkvt = nc.dram_tensor("wqkvt", [D, 3 * HL * DK], dt.bfloat16,
                           kind="ExternalInput")
    wot = nc.dram_tensor("wot", [HL * DK, D], dt.bfloat16, kind="ExternalInput")
    cosb = nc.dram_tensor("cosb", [S, 64], dt.bfloat16, kind="ExternalInput")
    sinb = nc.dram_tensor("sinb", [S, 64], dt.bfloat16, kind="ExternalInput")
    tri = nc.dram_tensor("tri", [P, P], dt.bfloat16, kind="ExternalInput")
    iden = nc.dram_tensor("iden", [P, P], dt.bfloat16, kind="ExternalInput")
    y = nc.dram_tensor("y", [S, D], dt.float32, kind="ExternalOutput")


    with tile.TileContext(nc) as tc, ExitStack() as top:
        const_pool = top.enter_context(tc.tile_pool(name="const", bufs=1))
        tri_t = const_pool.tile([P, P], dt.bfloat16)
        nc.sync.dma_start(tri_t[:], tri[:])
        iden_t = const_pool.tile([P, P], dt.bfloat16)
        nc.sync.dma_start(iden_t[:], iden[:])

        persist = top.enter_context(tc.tile_pool(name="persist", bufs=1))
        qT = persist.tile([P, HL, S], dt.bfloat16)          # [d, h, s]
        kT = persist.tile([P, HL, S], dt.bfloat16)          # [d, h, s]
        vS = persist.tile([P, NSC, HL, DK + 2], dt.bfloat16)  # [s_in, sc, h, d+ones]


        # ones column for the denominator (cols 128.. of each v chunk)
        nc.vector.memset(vS[:, :, :, DK:], 1.0)

        from contextlib import nullcontext
        loop_cm = tc.For_i(0, repeat, 1) if repeat > 1 else nullcontext()
        with loop_cm:
            # -------- Phase A+B: q,k,v projection + RoPE + transposes ----
            # Two passes of 4 heads each; one x^T LDW feeds 3 matmuls
            # (q, k, v blocks of 512 = 4 heads).
            with ExitStack() as ctxA:
                cpool = ctxA.enter_context(tc.tile_pool(name="cossin", bufs=1))
                cos_t = cpool.tile([P, NSC, 64], dt.bfloat16)
                nc.gpsimd.dma_start(
                    cos_t[:], cosb[:].rearrange("(sc p) j -> p sc j", p=P))
                sin_t = cpool.tile([P, NSC, 64], dt.bfloat16)
                nc.gpsimd.dma_start(
                    sin_t[:], sinb[:].rearrange("(sc p) j -> p sc j", p=P))

                for g in range(2):                # head groups 0-3, 4-7
                    with ExitStack() as ctx2:
                        wpool = ctx2.enter_context(tc.tile_pool(name="wg", bufs=1))
                        xpool = ctx2.enter_context(tc.tile_pool(name="xa", bufs=2))
                        psA = ctx2.enter_context(
                            tc.tile_pool(name="psA", bufs=2, space="PSUM"))
                        psT = ctx2.enter_context(
                            tc.tile_pool(name="psT", bufs=2, space="PSUM"))
                        rpool = ctx2.enter_context(tc.tile_pool(name="rope", bufs=2))

                        w_c = [wpool.tile([P, 1536], dt.bfloat16,
                                          tag=f"w{ic}", name=f"w{ic}")
                               for ic in range(NIC)]
                        xt_first = xpool.tile([P, NIC, 256], dt.bfloat16, tag="xt")
                        nc.sync.dma_start(xt_first[:], xt[:, 0, :, :])
                        for ic in range(NIC):
                            nc.gpsimd.dma_start(
                                w_c[ic][:],
                                wqkvt[ic * P:(ic + 1) * P,
                                      g * 1536:(g + 1) * 1536])

                        for so in range(S // 256):    # x chunks of 256
                            if so == 0:
                                xt_t = xt_first
                            else:
                                xt_t = xpool.tile([P, NIC, 256], dt.bfloat16,
                                                  tag="xt")
                                nc.sync.dma_start(xt_t[:], xt[:, so, :, :])
                            for ss in range(2):       # s-subchunk of 128
                                sc = so * 2 + ss
                                pq = [psA.tile([P, 512], dt.float32, tag=f"pa{j}",
                                               name=f"pa{j}")
                                      for j in range(3)]
                                for ic in range(NIC):
                                    lw = xt_t[:, ic, ss * 128:(ss + 1) * 128]
                                    for j in range(3):
                                        nc.tensor.matmul(
                                            pq[j][:], lw,
                                            w_c[ic][:, j * 512:(j + 1) * 512],
                                            start=(ic == 0), stop=(ic == NIC - 1))
                                # v block straight to vS (natural [s, d] layout)
                                nc.any.tensor_copy(
                                    out=vS[:, sc, g * 4:(g + 1) * 4, 0:DK],
                                    in_=pq[2][:].rearrange("p (t d) -> p t d", d=P))
                                # q,k blocks: copy, RoPE, transpose
                                qk_s = rpool.tile([P, 8, P], dt.bfloat16, tag="qks")
                                for j in range(2):
                                    nc.any.tensor_copy(
                                        out=qk_s[:, j * 4:(j + 1) * 4, :],
                                        in_=pq[j][:].rearrange(
                                            "p (t d) -> p t d", d=P))
                                # RoPE (evens-first: E=cols 0:64, O=cols 64:128)
                                E = qk_s[:, :, 0:64]
                                O = qk_s[:, :, 64:128]
                                cB = cos_t[:, sc, None, :].to_broadcast((P, 8, 64))
                                sB = sin_t[:, sc, None, :].to_broadcast((P, 8, 64))
                                t1 = rpool.tile([P, 8, 64], dt.bfloat16, tag="t1")
                                t2 = rpool.tile([P, 8, 64], dt.bfloat16, tag="t2")
                                rot = rpool.tile([P, 8, P], dt.bfloat16, tag="rot")
                                if "rope" in ablate:
                                    nc.vector.tensor_copy(rot[:], qk_s[:])
                                else:
                                    nc.vector.tensor_mul(out=t1[:], in0=E, in1=cB)
                                    nc.vector.tensor_mul(out=t2[:], in0=O, in1=sB)
                                    nc.vector.tensor_sub(out=rot[:, :, 0:64],
                                                         in0=t1[:], in1=t2[:])
                                    nc.vector.tensor_mul(out=t1[:], in0=O, in1=cB)
                                    nc.vector.tensor_mul(out=t2[:], in0=E, in1=sB)
                                    nc.vector.tensor_add(out=rot[:, :, 64:128],
                                                         in0=t1[:], in1=t2[:])
                                for tt in range(8 if "trans" not in ablate else 0):
                                    pst = psT.tile([P, P], dt.bfloat16, tag="ptr")
                                    nc.tensor.transpose(pst[:], rot[:, tt, :],
                                                        iden_t[:])
                                    dst = qT if tt < 4 else kT
                                    nc.any.tensor_copy(
                                        out=dst[:, g * 4 + tt % 4,
                                                sc * 128:(sc + 1) * 128],
                                        in_=pst[:])

            # ---------------- Phase C: attention ---------------------------
            with ExitStack() as ctxCD:
                opool = ctxCD.enter_context(tc.tile_pool(name="oTp", bufs=1))
                oTt = [opool.tile([P, HL, 512], dt.bfloat16, tag=f"oT{t}",
                                  name=f"oT{t}")
                       for t in range(S // 512)]  # attn out^T [d, h, s] per t
                # prefetch Wo during attention
                wpool = ctxCD.enter_context(tc.tile_pool(name="wo", bufs=1))
                wo_c = [wpool.tile([P, D], dt.bfloat16, tag=f"wo{hh}",
                                   name=f"wo{hh}") for hh in range(HL)]
                dma_engs = [nc.gpsimd, nc.gpsimd]
                for hh in range(HL):
                    dma_engs[hh % 2].dma_start(wo_c[hh][:],
                                               wot[hh * P:(hh + 1) * P, :])
                ctx = ctxCD.enter_context(ExitStack())
                psS = ctx.enter_context(tc.tile_pool(name="psS", bufs=2, space="PSUM"))
                psO = ctx.enter_context(tc.tile_pool(name="psO", bufs=1, space="PSUM"))
                psT2 = ctx.enter_context(tc.tile_pool(name="psT2", bufs=1, space="PSUM"))
                apool = ctx.enter_context(tc.tile_pool(name="attn", bufs=4))
                npool = ctx.enter_context(tc.tile_pool(name="norm", bufs=2))

                SQT = 512                         # sq tile width
                NSS = SQT // 128                  # 4 sq-subchunks / tile
                psD = ctx.enter_context(
                    tc.tile_pool(name="psD", bufs=1, space="PSUM"))
                ypool = ctx.enter_context(tc.tile_pool(name="ysb", bufs=2))

                def emit_d(t, sl):
                    """Output-projection chunk for s rows [512t+128sl, +128).

                    Emitted interleaved with the next tile-row's attention so
                    its matmuls fill PE idle while ScalarE runs exp.
                    """
                    if "d" not in phases:
                        return
                    oT = oTt[t]
                    sc = t * NSS + sl
                    y_sb = ypool.tile([P, D], dt.float32, tag="ysb")
                    for j in range(4):
                        yp = psD.tile([P, 512], dt.float32, tag="pd", name="pd")
                        for h in range(HL):
                            nc.tensor.matmul(
                                yp[:], oT[:, h, sl * 128:(sl + 1) * 128],
                                wo_c[h][:, j * 512:(j + 1) * 512],
                                start=(h == 0), stop=(h == HL - 1))
                        nc.any.tensor_copy(out=y_sb[:, j * 512:(j + 1) * 512],
                                           in_=yp[:])
                    nc.sync.dma_start(y[sc * P:(sc + 1) * P, :], y_sb[:])

                for t in range(S // SQT):
                    oT = oTt[t]
                    for h in range(HL if "c" in phases else 0):
                        out_ps = [psO.tile([P, DK + 2], dt.float32,
                                           tag=f"outp{b}", name=f"outp{b}")
                                  for b in range(NSS)]
                        nkv = NSS * (t + 1)
                        for c in range(nkv):
                            r = c - NSS * t       # >=0: diagonal-region chunk
                            sq_off = 128 * r if r > 0 else 0
                            width = SQT - sq_off
                            q0 = t * SQT + sq_off
                            sc_ps = psS.tile([P, SQT], dt.float32, tag="sc")
                            nc.tensor.matmul(
                                sc_ps[:, :width],
                                kT[:, h, c * 128:(c + 1) * 128],
                                qT[:, h, q0:q0 + width],
                                start=True, stop=True)
                            pt = apool.tile([P, SQT], dt.bfloat16, tag="pt")
                            nc.scalar.activation(pt[:, :width], sc_ps[:, :width],
                                                 AF.Exp, scale=SCALE)
                            if r >= 0:
                                pt_m = apool.tile([P, P], dt.bfloat16, tag="ptm")
                                nc.vector.tensor_mul(out=pt_m[:],
                                                     in0=pt[:, 0:128], in1=tri_t[:])
                            for ss in range(max(r, 0), NSS):
                                lo = ss * 128 - sq_off
                                lhsT = pt_m[:] if (r >= 0 and ss == r) \
                                    else pt[:, lo:lo + 128]
                                nc.tensor.matmul(
                                    out_ps[ss], lhsT,
                                    vS[:, c, h, :],
                                    start=(c == 0), stop=(c == NSS * t + ss))
                        # evict accumulators fast (frees PSUM), then normalize
                        for ss in range(NSS):
                            raw = npool.tile([P, DK + 2], dt.float32, tag="raw")
                            nc.vector.tensor_copy(raw[:], out_ps[ss][:])
                            rec = npool.tile([P, 1], dt.float32, tag="rec")
                            nc.vector.reciprocal(rec[:], raw[:, DK:DK + 1])
                            ob = npool.tile([P, DK], dt.bfloat16, tag="ob")
                            nc.vector.tensor_scalar_mul(ob[:], raw[:, 0:DK],
                                                        rec[:])
                            pst = psT2.tile([P, P], dt.bfloat16, tag="ptr2")
                            nc.tensor.transpose(pst[:], ob[:], iden_t[:])
                            nc.vector.tensor_copy(oT[:, h, ss * 128:(ss + 1) * 128],
                                                  pst[:])
                        # interleave previous tile-row's output projection
                        if t >= 1 and h % 2 == 1:
                            emit_d(t - 1, h // 2)
                    if t == S // SQT - 1:
                        for sl in range(NSS):
                            emit_d(t, sl)

    nc.finalize()
    return nc


def _make_runner(nc, n_cores=NCORES):
    import jax
    from jax.sharding import Mesh, PartitionSpec
    from jax.experimental.shard_map import shard_map

    import concourse.mybir as mybir_mod
    import concourse.mybir as mybir
    from concourse import bass2jax
    from concourse.bass2jax import _bass_exec_p, install_neuronx_cc_hook

    install_neuronx_cc_hook()
    in_names, out_names, out_avals = [], [], []
    partition_name = nc.partition_id_tensor.name if nc.partition_id_tensor else None
    for alloc in nc.m.functions[0].allocations:
        if not isinstance(alloc, mybir_mod.MemoryLocationSet):
            continue
        name = alloc.memorylocations[0].name
        if alloc.kind == "ExternalInput":
            if name != partition_name:
                in_names.append(name)
        elif alloc.kind == "ExternalOutput":
            out_names.append(name)
            out_avals.append(jax.core.ShapedArray(
                tuple(alloc.tensor_shape), mybir.dt.np(alloc.dtype)))
    n_params = len(in_names)
    all_in_names = list(in_names) + list(out_names)
    if partition_name is not None:
        all_in_names.append(partition_name)

    def _body(*args):
        operands = list(args)
        if partition_name is not None:
            operands.append(bass2jax.partition_id_tensor())
        outs = _bass_exec_p.bind(
            *operands,
            out_avals=tuple(out_avals),
            in_names=tuple(all_in_names),
            out_names=tuple(out_names),
            lowering_input_output_aliases=(),
            sim_require_finite=True,
            sim_require_nnan=True,
            nc=nc,
        )
        return tuple(outs)

    devices = jax.devices()[:n_cores]
    mesh = Mesh(np.asarray(devices), ("core",))
    in_specs = (PartitionSpec("core"),) * (n_params + len(out_names))
    out_specs = (PartitionSpec("core"),) * len(out_names)
    fn = jax.jit(shard_map(_body, mesh=mesh, in_specs=in_specs,
                           out_specs=out_specs, check_rep=False))
    zero_outs = [np.zeros((n_cores * a.shape[0],) + tuple(a.shape[1:]), a.dtype)
                 for a in out_avals]
    return fn, in_names, out_names, zero_outs


def _get_runner(repeat=1):
    key = (repeat, _PHASES, tuple(sorted(_ABLATE)))
    if key not in _RUNNER_CACHE:
        nc = _build_nc(repeat)
        _RUNNER_CACHE[key] = _make_runner(nc)
    return _RUNNER_CACHE[key]


def _prep_in_maps(x, Wq, Wk, Wv, Wo, cos, sin):
    x = np.asarray(x, dtype=np.float32)
    Wq = np.asarray(Wq, dtype=np.float32)
    Wk = np.asarray(Wk, dtype=np.float32)
    Wv = np.asarray(Wv, dtype=np.float32)
    Wo = np.asarray(Wo, dtype=np.float32)
    cos = np.asarray(cos, dtype=np.float32)
    sin = np.asarray(sin, dtype=np.float32)

    # evens-first permutation of the head dim for q/k (RoPE pair layout)
    perm = np.concatenate([np.arange(0, DK, 2), np.arange(1, DK, 2)])
    rows = (np.arange(H)[:, None] * DK + perm[None, :]).reshape(-1)
    Wq_p = Wq[rows]
    Wk_p = Wk[rows]

    cosb = cos.astype(BF16)
    sinb = sin.astype(BF16)
    tri = (np.arange(P)[:, None] <= np.arange(P)[None, :]).astype(BF16)
    iden = np.eye(P, dtype=np.float32).astype(BF16)

    in_maps = []
    for c in range(NCORES):
        b, hh = divmod(c, 2)
        osl = slice(hh * HL * DK, (hh + 1) * HL * DK)
        xt = x[b].T.reshape(NIC, P, S // 256, 256).transpose(1, 2, 0, 3)
        g0 = slice(hh * HL * DK, hh * HL * DK + 4 * DK)
        g1 = slice(hh * HL * DK + 4 * DK, (hh + 1) * HL * DK)
        wqkvt = np.concatenate(
            [Wq_p[g0].T, Wk_p[g0].T, Wv[g0].T,
             Wq_p[g1].T, Wk_p[g1].T, Wv[g1].T], axis=1)
        in_maps.append({
            "xt": np.ascontiguousarray(xt).astype(BF16),
            "wqkvt": wqkvt.astype(BF16),
            "wot": np.ascontiguousarray(Wo[:, osl].T).astype(BF16),
            "cosb": cosb,
            "sinb": sinb,
            "tri": tri,
            "iden": iden,
        })
    return in_maps


def _run(in_maps, repeat=1):
    import jax

    fn, in_names, out_names, zero_outs = _get_runner(repeat)
    concat_in = [np.concatenate([m[name] for m in in_maps], axis=0)
                 for name in in_names]
    out_arrs = fn(*concat_in, *zero_outs)
    yname = out_names.index("y")
    yall = np.asarray(out_arrs[yname]).reshape(NCORES, S, D)
    return yall


def kernel(x, Wq, Wk, Wv, Wo, cos, sin):
    in_maps = _prep_in_maps(x, Wq, Wk, Wv, Wo, cos, sin)
    yall = _run(in_maps)
    out = np.empty((B, S, D), dtype=np.float32)
    for b in range(B):
        out[b] = yall[2 * b] + yall[2 * b + 1]
    return out
